# revision 43
# baseline (speedup 1.0000x reference)
"""Causal self-attention (B=4, T=2048, D=1024, H=16, HD=64) on 8 TRN2 NeuronCores.

Sharding: core = (batch b in 0..3, head-group g in 0..1) -> data parallel on B,
tensor parallel over heads (8 heads per core). Each core computes a partial
out-projection for its head group; the host sums the pair of partials per batch
(the TP all-reduce done at unshard time).

Device kernel (per core), designed to keep the PE continuously busy (full
2.4 GHz p-state) by software-pipelining all phases into one dense PE stream:

  phase B   : xT via PE transposes interleaved with the v projection, with
              q0/k0 projection chunks interleaved as they become ready.
  attention : per head-pair g (heads 2g, 2g+1 share a qT/kT tile):
              S^T tiles as [128,1024] two-bank psum groups (2 tk tiles per
              exp), causal mask added by an identity matmul accumulating into
              the S psum (no DVE mask add), exp on ACT reading both banks,
              AV+Z fused (ones column -> row 64 = Z). The q/k projections for
              pair g+1 are emitted as PE filler between attention matmuls.
              Normalization per block: reciprocal_approx_fast on the psum Z
              row, partition_broadcast on GpSimd, in-place DVE multiply.
  tail      : out projection (dense PE) + bias drain + DMA out.

q/k tiles, v tiles and exp(S) are bf16 (psum accumulation stays f32);
x^T / y^T / weights are float32r. Strictly-upper tk-tiles are skipped
(half the attention FLOPs); diagonal tiles get the transposed mask block.
"""

import os
import numpy as np

import concourse.bass as bass
import concourse.tile as tile
from concourse import bacc, mybir
import concourse.bass_utils as bass_utils
from concourse.masks import make_identity

F32 = mybir.dt.float32
F32R = mybir.dt.float32r
BF16 = mybir.dt.bfloat16
AF = mybir.ActivationFunctionType
ALU = mybir.AluOpType

B, T, D, H = 4, 2048, 1024, 16
HD = D // H          # 64
G = 2                # head groups (TP degree)
HPG = H // G         # 8 heads per core
DG = HPG * HD        # 512 local qkv dims per core
NT = T // 128        # 16 row tiles
ND = D // 128        # 8 contraction tiles
NC = T // 512        # 4 tq chunks
NK = DG // 128       # 4 local-dim tiles (out-proj contraction)
NP = HPG // 2        # 4 head pairs
VW = HD + 1          # 65: v columns per head incl. ones column

_cached = {}


def _build():
    nc = bacc.Bacc("TRN2", target_bir_lowering=False, debug=False, num_devices=8)

    x_d = nc.dram_tensor("x", [T, D], F32R, kind="ExternalInput")
    wq_d = nc.dram_tensor("wq", [D, DG], F32R, kind="ExternalInput")
    wk_d = nc.dram_tensor("wk", [D, DG], F32R, kind="ExternalInput")
    wv_d = nc.dram_tensor("wv", [D, DG], F32R, kind="ExternalInput")
    wo_d = nc.dram_tensor("wo", [DG, D], F32R, kind="ExternalInput")
    bq_d = nc.dram_tensor("bq", [DG], F32, kind="ExternalInput")
    bk_d = nc.dram_tensor("bk", [DG], F32, kind="ExternalInput")
    bv_d = nc.dram_tensor("bv", [DG], F32R, kind="ExternalInput")
    bo_d = nc.dram_tensor("bo", [D], F32R, kind="ExternalInput")
    out_d = nc.dram_tensor("out", [T, D], F32, kind="ExternalOutput")

    with tile.TileContext(nc) as tc:
        with nc.allow_low_precision(reason="bf16 qkv/probs, fp32 psum accum"):
            _emit(nc, tc, x_d, wq_d, wk_d, wv_d, wo_d, bq_d, bk_d, bv_d, bo_d,
                  out_d)
    nc.finalize()
    return nc


def _emit(nc, tc, x_d, wq_d, wk_d, wv_d, wo_d, bq_d, bk_d, bv_d, bo_d,
          out_d):
    from contextlib import ExitStack
    ctx = ExitStack()
    with ctx:
        # ---------------- persistent pools ----------------
        const_p = ctx.enter_context(tc.tile_pool(name="const", bufs=1))
        qk_p = ctx.enter_context(tc.tile_pool(name="qk", bufs=1))
        vp_p = ctx.enter_context(tc.tile_pool(name="vp", bufs=1))
        yt_p = ctx.enter_context(tc.tile_pool(name="yt", bufs=1))
        pt_p = ctx.enter_context(tc.tile_pool(name="pt", bufs=3))
        zbc_p = ctx.enter_context(tc.tile_pool(name="zbc", bufs=2))
        wblk_p = ctx.enter_context(tc.tile_pool(name="wblk", bufs=2))
        # qk-projection psum: spans phase B and the attention filler stream
        qkproj_ps = ctx.enter_context(
            tc.tile_pool(name="qkproj_ps", bufs=2, space="PSUM"))

        # pool open order mirrors reverse close order (stack allocator):
        # xt (pair-2 end) <- wv (pair-0 end) <- xtmp (phase-B end)
        xt_es = ExitStack()
        xt_p = xt_es.enter_context(tc.tile_pool(name="xt", bufs=1))
        # x^T as one tile [128 dpart, 8 dtile, 2048 t]
        xt = xt_p.tile([128, ND, T], F32R, tag="xt", name="xt")
        wv_es = ExitStack()
        wv_p = wv_es.enter_context(tc.tile_pool(name="wv", bufs=1))

        xtmp_es = ExitStack()
        xtmp_p = xtmp_es.enter_context(tc.tile_pool(name="xtmp", bufs=4))

        # identities (f32 for mask transpose, f32r for x transpose + mask add)
        ident32 = const_p.tile([128, 128], F32, tag="ident32")
        make_identity(nc, ident32[:])
        identr = const_p.tile([128, 128], F32R, tag="identr")
        nc.vector.tensor_copy(identr[:], ident32[:])

        # q/k bias columns [128, 8]: cols 0-3 = bq tiles, 4-7 = bk tiles
        bqk = const_p.tile([128, 8], F32, tag="bqk")
        nc.sync.dma_start(bqk[:, 0:NK], bq_d[:].rearrange("(f p) -> p f", p=128))
        nc.sync.dma_start(bqk[:, NK:2 * NK], bk_d[:].rearrange("(f p) -> p f", p=128))

        # broadcast bv -> [128, 512] and bout -> [128, 1024] via e0 matmuls
        bv_bc = const_p.tile([128, DG], F32, tag="bv_bc")
        bo_bc = const_p.tile([128, D], F32, tag="bo_bc")
        # prewarm the Exp activation table while ACT is idle (the lazy load
        # otherwise stalls the first exps of the attention phase)
        actwarm = const_p.tile([1, 8], F32, tag="actwarm")
        nc.vector.memset(actwarm[:], 0.0)
        nc.scalar.activation(actwarm[:], actwarm[:], AF.Exp)
        with (
            tc.tile_pool(name="brow", bufs=1) as brow_p,
            tc.tile_pool(name="bc_ps", bufs=3, space="PSUM") as bc_ps,
        ):
            # e0 pattern [128, 128]: row 0 ones, else zeros
            e0 = brow_p.tile([128, 128], F32R, tag="e0")
            nc.vector.memset(e0[:].bitcast(F32), 0.0)
            nc.vector.memset(e0[0:1, :].bitcast(F32), 1.0)
            bvrow = brow_p.tile([128, DG], F32R, tag="bvrow")
            nc.vector.memset(bvrow[:].bitcast(F32), 0.0)
            nc.sync.dma_start(bvrow[0:1, :], bv_d[:].rearrange("(o n) -> o n", o=1))
            pb = bc_ps.tile([128, DG], F32)
            nc.tensor.matmul(pb[:], e0[:], bvrow[:], start=True, stop=True)
            nc.vector.tensor_copy(bv_bc[:], pb[:])
            for oc in range(2):
                borow = brow_p.tile([128, 512], F32R, tag="borow",
                                    name="borow")
                nc.vector.memset(borow[:].bitcast(F32), 0.0)
                nc.sync.dma_start(
                    borow[0:1, :],
                    bo_d[512 * oc:512 * (oc + 1)].rearrange(
                        "(o n) -> o n", o=1))
                pob = bc_ps.tile([128, 512], F32, tag="bo")
                nc.tensor.matmul(pob[:], e0[:], borow[:],
                                 start=True, stop=True)
                nc.vector.tensor_copy(bo_bc[:, 512 * oc:512 * (oc + 1)], pob[:])

        # x staging: each of the first tiles as two half-row DMAs to halve
        # its arrival latency; the transposes start as soon as t0 lands
        xtmps = {}
        for t in range(4):
            xtmp = xtmp_p.tile([128, D], F32R, tag="xtmp", name="xtmp")
            for hh in range(2):
                nc.sync.dma_start(
                    xtmp[:, 512 * hh:512 * (hh + 1)],
                    x_d[128 * t:128 * (t + 1), 512 * hh:512 * (hh + 1)])
            xtmps[t] = xtmp

        # v' tiles [128, 8*65] bf16, ones columns set via strided memset
        vp = []
        for t in range(NT):
            vt = vp_p.tile([128, HPG * VW], BF16, tag=f"vp{t}", name=f"vp{t}")
            nc.vector.memset(
                vt[:].rearrange("p (h c) -> p h c", h=HPG)[:, :, HD:HD + 1], 1.0)
            vp.append(vt)

        # qkT tiles [128, 2048] bf16: 0-3 = qT (head pairs), 4-7 = kT
        qk = [qk_p.tile([128, T], BF16, tag=f"qk{f}", name=f"qk{f}")
              for f in range(2 * NK)]

        # yT tiles [128, 2048] bf16 (out-proj stationary)
        yt = [yt_p.tile([128, T], BF16, tag=f"yt{k}", name=f"yt{k}")
              for k in range(NK)]

        # ---------------- qk projection filler generator ----------------
        # DMAs are emitted eagerly at creation; the returned generator emits
        # the matmuls in ~4-matmul units so they can be interleaved into the
        # attention PE stream of the previous pair.
        def qk_filler(g):
            tiles = {}
            for f in (g, NK + g):  # q then k weight block
                src = wq_d if f < NK else wk_d
                fc = f % NK
                wblk = wblk_p.tile([128, D], F32R,
                                   tag="wq" if f < NK else "wk", name="wblk")
                nc.sync.dma_start(
                    wblk[:].rearrange("p (dt c) -> p dt c", dt=ND),
                    src[:, 128 * fc:128 * (fc + 1)].rearrange(
                        "(dt p) c -> p dt c", p=128))
                tiles[f] = wblk

            def units():
                for c in range(NC):
                    for f in (g, NK + g):
                        wblk = tiles[f]
                        pq = qkproj_ps.tile([128, 512], F32, tag="pq")
                        for dh in range(2):
                            for d in range(4 * dh, 4 * dh + 4):
                                nc.tensor.matmul(
                                    pq[:], wblk[:, 128 * d:128 * (d + 1)],
                                    xt[:, d, 512 * c:512 * (c + 1)],
                                    start=(d == 0), stop=(d == ND - 1),
                                    skip_group_check=True)
                            if dh == 0:
                                yield
                        nc.vector.tensor_scalar(
                            qk[f][:, 512 * c:512 * (c + 1)], pq[:],
                            bqk[:, f:f + 1], None, ALU.add)
                        yield

            return units()

        # ---------------- phase B: xT + v + q0/k0 ----------------
        with (
            tc.tile_pool(name="t_ps", bufs=2, space="PSUM") as t_ps,
            tc.tile_pool(name="v_ps", bufs=2, space="PSUM") as v_ps,
        ):
            wv_sb = []
            for d in range(ND):
                wt = wv_p.tile([128, DG], F32R, tag=f"wv{d}", name=f"wvt{d}")
                nc.sync.dma_start(wt[:], wv_d[128 * d:128 * (d + 1), :])
                wv_sb.append(wt)
            f0 = qk_filler(0)

            def v_proj(t):
                pv = v_ps.tile([128, DG], F32, tag="pv")
                for d in range(ND):
                    nc.tensor.matmul(pv[:], xt[:, d, 128 * t:128 * (t + 1)],
                                     wv_sb[d][:], start=(d == 0),
                                     stop=(d == ND - 1))
                nc.vector.tensor_tensor(
                    vp[t][:].rearrange("p (h c) -> p h c", h=HPG)[:, :, 0:HD],
                    pv[:].rearrange("p (h c) -> p h c", h=HPG),
                    bv_bc[:].rearrange("p (h c) -> p h c", h=HPG),
                    ALU.add)

            for t in range(NT):
                if t < 4:
                    xtmp = xtmps[t]
                else:
                    xtmp = xtmp_p.tile([128, D], F32R, tag="xtmp", name="xtmp")
                    for hh in range(2):
                        nc.sync.dma_start(
                            xtmp[:, 512 * hh:512 * (hh + 1)],
                            x_d[128 * t:128 * (t + 1), 512 * hh:512 * (hh + 1)])
                # v of older tiles first: always-ready PE work that covers
                # the wait for x tile t's DMA; v of tiles 12-15 is deferred
                # into pair 0's attention window
                if 2 <= t and t - 2 < NT - 4:
                    v_proj(t - 2)
                for half in range(2):
                    tp = t_ps.tile([128, 512], F32R, tag="tp")
                    for dd in range(4):
                        d = 4 * half + dd
                        nc.tensor.transpose(
                            tp[:, 128 * dd:128 * (dd + 1)],
                            xtmp[:, 128 * d:128 * (d + 1)], identr[:])
                    nc.vector.tensor_copy(
                        xt[:, 4 * half:4 * half + 4, 128 * t:128 * (t + 1)],
                        tp[:].rearrange("p (a b) -> p a b", a=4))
                if t % 4 == 3:
                    # q0/k0 chunk (t-3)//4 just became computable: 4 units
                    for _ in range(4):
                        next(f0, None)
            for _ in f0:
                pass
        xtmp_es.close()

        # deferred v projection for tiles 12-15, emitted as pair-0 filler
        def v_filler():
            for t in range(NT - 4, NT):
                pv = qkproj_ps.tile([128, DG], F32, tag="pq", name="pv")
                for d in range(ND):
                    nc.tensor.matmul(pv[:], xt[:, d, 128 * t:128 * (t + 1)],
                                     wv_sb[d][:], start=(d == 0),
                                     stop=(d == ND - 1),
                                     skip_group_check=True)
                    if d == 3:
                        yield
                nc.vector.tensor_tensor(
                    vp[t][:].rearrange("p (h c) -> p h c", h=HPG)[:, :, 0:HD],
                    pv[:].rearrange("p (h c) -> p h c", h=HPG),
                    bv_bc[:].rearrange("p (h c) -> p h c", h=HPG),
                    ALU.add)
                yield

        # ---------------- attention ----------------
        def attention_pair(g, filler, rate=2, budget=None):
            qt = qk[g]
            kt = qk[NK + g]
            consumed = [0]

            def filler_step(n=1):
                for _ in range(n):
                    if next(filler, None) is not None or True:
                        consumed[0] += 1

            slot = 0
            for c in range(NC):
                JJ = 2 * c + 2
                pys = [py_ps.tile([VW, 512], F32, tag="py", name=f"py{g}{c}{x}")
                       for x in range(2)]

                def emit_avs(ent, last):
                    xi, jj, pt = ent
                    hl = 2 * g + xi
                    for half in range(2):
                        j = 2 * jj + half
                        # diagonal tiles: columns < 128m are fully masked and
                        # were never computed; skip them (psum cols keep their
                        # start-reset value from the first full-width matmul)
                        m = j - 4 * c
                        lo = 128 * m if m > 0 else 0
                        nc.tensor.matmul(
                            pys[xi][:, lo:512], vp[j][:, VW * hl:VW * (hl + 1)],
                            pt[:, 512 * half + lo:512 * (half + 1)],
                            start=(jj == 0 and half == 0),
                            stop=(last and half == 1),
                            skip_group_check=True)
                    if last:
                        qrow = xi * 64
                        ysl = yt[g][qrow:qrow + HD, 512 * c:512 * (c + 1)]
                        zr = zbc_p.tile([1, 512], F32, tag="zr", name="zr")
                        if g == NP - 1 and c == NC - 1:
                            # last chunk: ACT is done with exps; keep the
                            # critical normalize chain off the busy DVE queue
                            nc.scalar.activation(ysl, pys[xi][0:64, :], AF.Copy)
                            nc.scalar.activation(zr[:], pys[xi][64:65, :],
                                                 AF.Copy)
                        else:
                            nc.vector.tensor_copy(ysl, pys[xi][0:64, :])
                            nc.vector.tensor_copy(zr[:], pys[xi][64:65, :])
                        zrr = zbc_p.tile([1, 512], F32, tag="zrr", name="zrr")
                        nc.vector.reciprocal_approx_fast(zrr[:], zr[:])
                        zbc = zbc_p.tile([128, 512], F32, tag="zbc")
                        nc.gpsimd.partition_broadcast(
                            zbc[:, :], zrr[:], channels=128)
                        nc.vector.tensor_tensor(
                            ysl, ysl, zbc[qrow:qrow + HD, :], ALU.mult)

                prev = None
                for jj in range(JJ):
                    for xi, qrow in ((0, 0), (1, 64)):
                        st = s_ps.tile([128, 1024], F32, tag="s")
                        for half in range(2):
                            j = 2 * jj + half
                            m = j - 4 * c
                            diag = m >= 0
                            # columns < 128m are fully masked: don't compute
                            # them (the trimmed AV never reads them)
                            lo = 128 * m if diag else 0
                            nc.tensor.matmul(
                                st[:, 512 * half + lo:512 * (half + 1)],
                                kt[qrow:qrow + HD, 128 * j:128 * (j + 1)],
                                qt[qrow:qrow + HD,
                                   512 * c + lo:512 * (c + 1)],
                                start=True, stop=True,
                                skip_group_check=True)
                        pt = pt_p.tile([128, 1024], BF16, tag="pt")
                        if 2 * jj >= 4 * c:
                            # diagonal pair: exp only the computed column
                            # ranges of each half (the gap was never written)
                            lo0 = 128 * (2 * jj - 4 * c)
                            nc.scalar.activation(
                                pt[:, lo0:512], st[:, lo0:512],
                                AF.Exp, scale=0.125)
                            nc.scalar.activation(
                                pt[:, 512 + lo0 + 128:1024],
                                st[:, 512 + lo0 + 128:1024],
                                AF.Exp, scale=0.125)
                            # zero the above-diagonal triangle of the leading
                            # 128-col block of each half: keep where
                            # col_local - p >= 0 (tq >= tk)
                            for half in range(2):
                                lo = lo0 + 128 * half
                                nc.gpsimd.affine_select(
                                    out=pt[:, 512 * half + lo:
                                           512 * half + lo + 128],
                                    in_=pt[:, 512 * half + lo:
                                           512 * half + lo + 128],
                                    compare_op=ALU.is_ge, fill=0.0,
                                    base=0, channel_multiplier=-1,
                                    pattern=[[1, 128]])
                        else:
                            nc.scalar.activation(pt[:], st[:], AF.Exp,
                                                 scale=0.125)
                        if prev is not None:
                            emit_avs(prev, prev[1] == JJ - 1)
                            if budget is not None:
                                if consumed[0] < budget(c):
                                    filler_step()
                            elif slot % rate == 0:
                                filler_step()
                            slot += 1
                        prev = (xi, jj, pt)
                emit_avs(prev, True)
                filler_step()

        # full out projection emitted as PE filler inside pair 3's window,
        # chunk-gated: tile t needs yt[3] chunk t//4 normalized first.
        # Final rows land in ohalf and DMA out immediately.
        def out_filler():
            # last-4 tiles first: yt[0..2] partials (no chunk gating needed)
            for t in range(NT - 4, NT):
                for oc in range(2):
                    po = qkproj_ps.tile([128, 512], F32, tag="pq", name="pof")
                    for k in range(NK - 1):
                        nc.tensor.matmul(
                            po[:], yt[k][:, 128 * t:128 * (t + 1)],
                            wo_sb[k][:, 512 * oc:512 * (oc + 1)],
                            start=(k == 0), stop=(k == NK - 2),
                            skip_group_check=True)
                    nc.vector.tensor_tensor(
                        ohalf[:, t, 512 * oc:512 * (oc + 1)], po[:],
                        bo_bc[:, 512 * oc:512 * (oc + 1)], ALU.add)
                    yield
            # the rest: complete rows, gated on pair-3 chunk completion, DMA
            # out immediately
            for t in range(NT - 4):
                for oc in range(2):
                    po = qkproj_ps.tile([128, 512], F32, tag="pq", name="pof")
                    for k in range(NK):
                        nc.tensor.matmul(
                            po[:], yt[k][:, 128 * t:128 * (t + 1)],
                            wo_sb[k][:, 512 * oc:512 * (oc + 1)],
                            start=(k == 0), stop=(k == NK - 1),
                            skip_group_check=True)
                    osl = ohalf[:, t, 512 * oc:512 * (oc + 1)]
                    nc.vector.tensor_tensor(
                        osl, po[:], bo_bc[:, 512 * oc:512 * (oc + 1)],
                        ALU.add)
                    nc.sync.dma_start(
                        out_d[128 * t:128 * (t + 1),
                              512 * oc:512 * (oc + 1)], osl.bitcast(F32))
                    yield

        def out_flush():
            # add the yt[3] term to the last-4 tiles: re-add the partial row
            # via an identity matmul so the drain is a plain ACT copy (DVE
            # stays off the critical tail)
            for t in range(NT - 4, NT):
                for oc in range(2):
                    po = qkproj_ps.tile([128, 512], F32, tag="pq", name="pofl")
                    nc.tensor.matmul(
                        po[:], yt[NK - 1][:, 128 * t:128 * (t + 1)],
                        wo_sb[NK - 1][:, 512 * oc:512 * (oc + 1)],
                        start=True, stop=False, skip_group_check=True)
                    osl = ohalf[:, t, 512 * oc:512 * (oc + 1)]
                    nc.tensor.matmul(
                        po[:], identr[:], osl,
                        start=False, stop=True, skip_group_check=True)
                    ofl = ofl_p.tile([128, 512], F32, tag="ofl", name="ofl")
                    nc.scalar.activation(ofl[:], po[:], AF.Copy)
                    nc.sync.dma_start(
                        out_d[128 * t:128 * (t + 1),
                              512 * oc:512 * (oc + 1)], ofl[:])

        with (
            tc.tile_pool(name="s_ps", bufs=2, space="PSUM") as s_ps,
            tc.tile_pool(name="y_ps", bufs=2, space="PSUM") as py_ps,
        ):
            for g in range(NP):
                if g == 0:
                    from itertools import chain
                    filler = chain(v_filler(), qk_filler(1))
                    attention_pair(g, filler, rate=2)
                elif g < NP - 1:
                    filler = qk_filler(g + 1)
                    attention_pair(g, filler, rate=2)
                else:
                    filler = out_filler()
                    attention_pair(g, filler, budget=lambda c: 8 + 8 * c)
                for _ in filler:
                    pass
                if g == NP - 1:
                    out_flush()
                if g == 0:
                    wv_es.close()
                if g == NP - 2:
                    # all qk projections emitted; xt no longer needed
                    xt_es.close()
                    # half-done out rows live where xt was
                    oh_p = ctx.enter_context(tc.tile_pool(name="ohalf", bufs=1))
                    # f32r so the flush identity-matmul may re-add it
                    ohalf = oh_p.tile([128, NT, D], F32R, tag="ohalf",
                                      name="ohalf")
                    ofl_p = ctx.enter_context(
                        tc.tile_pool(name="oflsh", bufs=2))
                    # out-proj weights: DMA f32 staging, cast to bf16
                    wo_p = ctx.enter_context(tc.tile_pool(name="wo", bufs=1))
                    wost_p = ctx.enter_context(
                        tc.tile_pool(name="wost", bufs=2))
                    wo_sb = []
                    for k in range(NK):
                        wst = wost_p.tile([128, D], F32R, tag="wost",
                                          name="wost")
                        nc.sync.dma_start(wst[:], wo_d[128 * k:128 * (k + 1), :])
                        wt = wo_p.tile([128, D], BF16, tag=f"wo{k}",
                                       name=f"wot{k}")
                        nc.vector.tensor_copy(wt[:], wst[:].bitcast(F32))
                        wo_sb.append(wt)



def kernel(x, attn_mask, Wqkv, bqkv, Wout, bout):
    if "nc" not in _cached:
        _cached["nc"] = _build()
    nc = _cached["nc"]

    x = np.ascontiguousarray(x, dtype=np.float32)
    Wqkv = np.ascontiguousarray(Wqkv, dtype=np.float32)
    bqkv = np.ascontiguousarray(bqkv, dtype=np.float32)
    Wout = np.ascontiguousarray(Wout, dtype=np.float32)
    bout = np.ascontiguousarray(bout, dtype=np.float32)
    zeros_bo = np.zeros_like(bout)
    in_maps = []
    for b in range(B):
        for g in range(G):
            s = slice(g * DG, (g + 1) * DG)
            in_maps.append({
                "x": np.ascontiguousarray(x[b]),
                "wq": np.ascontiguousarray(Wqkv[:, g * DG:(g + 1) * DG]),
                "wk": np.ascontiguousarray(Wqkv[:, D + g * DG:D + (g + 1) * DG]),
                "wv": np.ascontiguousarray(Wqkv[:, 2 * D + g * DG:2 * D + (g + 1) * DG]),
                "wo": np.ascontiguousarray(Wout[s, :]),
                "bq": np.ascontiguousarray(bqkv[g * DG:(g + 1) * DG]),
                "bk": np.ascontiguousarray(bqkv[D + g * DG:D + (g + 1) * DG]),
                "bv": np.ascontiguousarray(bqkv[2 * D + g * DG:2 * D + (g + 1) * DG]),
                "bo": bout if g == 0 else zeros_bo,
            })

    trace = bool(int(os.environ.get("BASS_ATTN_TRACE", "0")))
    res = bass_utils.run_bass_kernel_spmd(
        nc, in_maps, core_ids=list(range(B * G)), trace=trace)
    _cached["last_result"] = res

    out = np.empty((B, T, D), dtype=np.float32)
    for b in range(B):
        out[b] = res.results[2 * b]["out"] + res.results[2 * b + 1]["out"]
    return out


# revision 45
# speedup vs baseline: 1.0059x; 1.0059x over previous
"""Causal self-attention (B=4, T=2048, D=1024, H=16, HD=64) on 8 TRN2 NeuronCores.

Sharding: core = (batch b in 0..3, head-group g in 0..1) -> data parallel on B,
tensor parallel over heads (8 heads per core). Each core computes a partial
out-projection for its head group; the host sums the pair of partials per batch
(the TP all-reduce done at unshard time).

Device kernel (per core), designed to keep the PE continuously busy (full
2.4 GHz p-state) by software-pipelining all phases into one dense PE stream:

  phase B   : xT via PE transposes interleaved with the v projection, with
              q0/k0 projection chunks interleaved as they become ready.
  attention : per head-pair g (heads 2g, 2g+1 share a qT/kT tile):
              S^T tiles as [128,1024] two-bank psum groups (2 tk tiles per
              exp), exp on ACT reading both banks, causal triangle zeroed
              post-exp by affine_select on the idle GpSimd engine, AV+Z fused
              (ones column -> row 64 = Z). The q/k projections for pair g+1
              are emitted as PE filler between attention matmuls; pair 3's
              filler is the out projection itself (chunk-gated), results DMA
              out from SBUF as they complete. Normalization per block: Z row
              staged to SBUF, reciprocal_approx_fast, partition_broadcast on
              GpSimd, in-place DVE multiply.
  flush     : last 4 out tiles add the yt[3] term (partial rows re-added via
              an identity matmul) and drain on the idle ACT engine.

q/k tiles, v tiles, y^T and exp(S) are bf16 (psum accumulation stays f32);
x^T and weights are float32r. Strictly-upper tk-tiles are skipped and
diagonal tiles are column-trimmed (over half the attention FLOPs saved).
The attn_mask input is unused on device: the causal structure is synthesized
with affine_select, which matches the reference's additive -1e9 mask exactly.
"""

import os
import numpy as np

import concourse.bass as bass
import concourse.tile as tile
from concourse import bacc, mybir
import concourse.bass_utils as bass_utils
from concourse.masks import make_identity

F32 = mybir.dt.float32
F32R = mybir.dt.float32r
BF16 = mybir.dt.bfloat16
AF = mybir.ActivationFunctionType
ALU = mybir.AluOpType

B, T, D, H = 4, 2048, 1024, 16
HD = D // H          # 64
G = 2                # head groups (TP degree)
HPG = H // G         # 8 heads per core
DG = HPG * HD        # 512 local qkv dims per core
NT = T // 128        # 16 row tiles
ND = D // 128        # 8 contraction tiles
NC = T // 512        # 4 tq chunks
NK = DG // 128       # 4 local-dim tiles (out-proj contraction)
NP = HPG // 2        # 4 head pairs
VW = HD + 1          # 65: v columns per head incl. ones column

_cached = {}


def _build():
    nc = bacc.Bacc("TRN2", target_bir_lowering=False, debug=False, num_devices=8)

    x_d = nc.dram_tensor("x", [T, D], F32R, kind="ExternalInput")
    wq_d = nc.dram_tensor("wq", [D, DG], F32R, kind="ExternalInput")
    wk_d = nc.dram_tensor("wk", [D, DG], F32R, kind="ExternalInput")
    wv_d = nc.dram_tensor("wv", [D, DG], F32R, kind="ExternalInput")
    wo_d = nc.dram_tensor("wo", [DG, D], F32R, kind="ExternalInput")
    bq_d = nc.dram_tensor("bq", [DG], F32, kind="ExternalInput")
    bk_d = nc.dram_tensor("bk", [DG], F32, kind="ExternalInput")
    bv_d = nc.dram_tensor("bv", [DG], F32R, kind="ExternalInput")
    bo_d = nc.dram_tensor("bo", [D], F32R, kind="ExternalInput")
    out_d = nc.dram_tensor("out", [T, D], F32, kind="ExternalOutput")

    with tile.TileContext(nc) as tc:
        with nc.allow_low_precision(reason="bf16 qkv/probs, fp32 psum accum"):
            _emit(nc, tc, x_d, wq_d, wk_d, wv_d, wo_d, bq_d, bk_d, bv_d, bo_d,
                  out_d)
    nc.finalize()
    return nc


def _emit(nc, tc, x_d, wq_d, wk_d, wv_d, wo_d, bq_d, bk_d, bv_d, bo_d,
          out_d):
    from contextlib import ExitStack
    ctx = ExitStack()
    with ctx:
        # ---------------- persistent pools ----------------
        const_p = ctx.enter_context(tc.tile_pool(name="const", bufs=1))
        qk_p = ctx.enter_context(tc.tile_pool(name="qk", bufs=1))
        vp_p = ctx.enter_context(tc.tile_pool(name="vp", bufs=1))
        yt_p = ctx.enter_context(tc.tile_pool(name="yt", bufs=1))
        pt_p = ctx.enter_context(tc.tile_pool(name="pt", bufs=3))
        zbc_p = ctx.enter_context(tc.tile_pool(name="zbc", bufs=2))
        wblk_p = ctx.enter_context(tc.tile_pool(name="wblk", bufs=2))
        # qk-projection psum: spans phase B and the attention filler stream
        qkproj_ps = ctx.enter_context(
            tc.tile_pool(name="qkproj_ps", bufs=2, space="PSUM"))

        # pool open order mirrors reverse close order (stack allocator):
        # xt (pair-2 end) <- wv (pair-0 end) <- xtmp (phase-B end)
        xt_es = ExitStack()
        xt_p = xt_es.enter_context(tc.tile_pool(name="xt", bufs=1))
        # x^T as one tile [128 dpart, 8 dtile, 2048 t]
        xt = xt_p.tile([128, ND, T], F32R, tag="xt", name="xt")
        wv_es = ExitStack()
        wv_p = wv_es.enter_context(tc.tile_pool(name="wv", bufs=1))

        xtmp_es = ExitStack()
        xtmp_p = xtmp_es.enter_context(tc.tile_pool(name="xtmp", bufs=4))

        # identities (f32 for mask transpose, f32r for x transpose + mask add)
        ident32 = const_p.tile([128, 128], F32, tag="ident32")
        make_identity(nc, ident32[:])
        identr = const_p.tile([128, 128], F32R, tag="identr")
        nc.vector.tensor_copy(identr[:], ident32[:])

        # q/k bias columns [128, 8]: cols 0-3 = bq tiles, 4-7 = bk tiles
        bqk = const_p.tile([128, 8], F32, tag="bqk")
        nc.sync.dma_start(bqk[:, 0:NK], bq_d[:].rearrange("(f p) -> p f", p=128))
        nc.sync.dma_start(bqk[:, NK:2 * NK], bk_d[:].rearrange("(f p) -> p f", p=128))

        # broadcast bv -> [128, 512] and bout -> [128, 1024] via e0 matmuls
        bv_bc = const_p.tile([128, DG], F32, tag="bv_bc")
        bo_bc = const_p.tile([128, D], F32, tag="bo_bc")
        # prewarm the Exp activation table while ACT is idle (the lazy load
        # otherwise stalls the first exps of the attention phase)
        actwarm = const_p.tile([1, 8], F32, tag="actwarm")
        nc.vector.memset(actwarm[:], 0.0)
        nc.scalar.activation(actwarm[:], actwarm[:], AF.Exp)
        with (
            tc.tile_pool(name="brow", bufs=1) as brow_p,
            tc.tile_pool(name="bc_ps", bufs=3, space="PSUM") as bc_ps,
        ):
            # e0 pattern [128, 128]: row 0 ones, else zeros
            e0 = brow_p.tile([128, 128], F32R, tag="e0")
            nc.vector.memset(e0[:].bitcast(F32), 0.0)
            nc.vector.memset(e0[0:1, :].bitcast(F32), 1.0)
            bvrow = brow_p.tile([128, DG], F32R, tag="bvrow")
            nc.vector.memset(bvrow[:].bitcast(F32), 0.0)
            nc.sync.dma_start(bvrow[0:1, :], bv_d[:].rearrange("(o n) -> o n", o=1))
            pb = bc_ps.tile([128, DG], F32)
            nc.tensor.matmul(pb[:], e0[:], bvrow[:], start=True, stop=True)
            nc.vector.tensor_copy(bv_bc[:], pb[:])
            for oc in range(2):
                borow = brow_p.tile([128, 512], F32R, tag="borow",
                                    name="borow")
                nc.vector.memset(borow[:].bitcast(F32), 0.0)
                nc.sync.dma_start(
                    borow[0:1, :],
                    bo_d[512 * oc:512 * (oc + 1)].rearrange(
                        "(o n) -> o n", o=1))
                pob = bc_ps.tile([128, 512], F32, tag="bo")
                nc.tensor.matmul(pob[:], e0[:], borow[:],
                                 start=True, stop=True)
                nc.vector.tensor_copy(bo_bc[:, 512 * oc:512 * (oc + 1)], pob[:])

        # x staging: each of the first tiles as two half-row DMAs to halve
        # its arrival latency; the transposes start as soon as t0 lands
        xtmps = {}

        def issue_x(t):
            xtmp = xtmp_p.tile([128, D], F32R, tag="xtmp", name="xtmp")
            for hh in range(2):
                nc.sync.dma_start(
                    xtmp[:, 512 * hh:512 * (hh + 1)],
                    x_d[128 * t:128 * (t + 1), 512 * hh:512 * (hh + 1)])
            xtmps[t] = xtmp

        for t in range(2):
            issue_x(t)
        wv_sb = []
        for d in range(ND):
            wt = wv_p.tile([128, DG], F32R, tag=f"wv{d}", name=f"wvt{d}")
            nc.sync.dma_start(wt[:], wv_d[128 * d:128 * (d + 1), :])
            wv_sb.append(wt)
        for t in range(2, 4):
            issue_x(t)

        # v' tiles [128, 8*65] bf16, ones columns set via strided memset
        vp = []
        for t in range(NT):
            vt = vp_p.tile([128, HPG * VW], BF16, tag=f"vp{t}", name=f"vp{t}")
            nc.vector.memset(
                vt[:].rearrange("p (h c) -> p h c", h=HPG)[:, :, HD:HD + 1], 1.0)
            vp.append(vt)

        # qkT tiles [128, 2048] bf16: 0-3 = qT (head pairs), 4-7 = kT
        qk = [qk_p.tile([128, T], BF16, tag=f"qk{f}", name=f"qk{f}")
              for f in range(2 * NK)]

        # yT tiles [128, 2048] bf16 (out-proj stationary)
        yt = [yt_p.tile([128, T], BF16, tag=f"yt{k}", name=f"yt{k}")
              for k in range(NK)]

        # ---------------- qk projection filler generator ----------------
        # DMAs are emitted eagerly at creation; the returned generator emits
        # the matmuls in ~4-matmul units so they can be interleaved into the
        # attention PE stream of the previous pair.
        def qk_filler(g):
            tiles = {}
            for f in (g, NK + g):  # q then k weight block
                src = wq_d if f < NK else wk_d
                fc = f % NK
                wblk = wblk_p.tile([128, D], F32R,
                                   tag="wq" if f < NK else "wk", name="wblk")
                nc.sync.dma_start(
                    wblk[:].rearrange("p (dt c) -> p dt c", dt=ND),
                    src[:, 128 * fc:128 * (fc + 1)].rearrange(
                        "(dt p) c -> p dt c", p=128))
                tiles[f] = wblk

            def units():
                for c in range(NC):
                    for f in (g, NK + g):
                        wblk = tiles[f]
                        pq = qkproj_ps.tile([128, 512], F32, tag="pq")
                        for dh in range(2):
                            for d in range(4 * dh, 4 * dh + 4):
                                nc.tensor.matmul(
                                    pq[:], wblk[:, 128 * d:128 * (d + 1)],
                                    xt[:, d, 512 * c:512 * (c + 1)],
                                    start=(d == 0), stop=(d == ND - 1),
                                    skip_group_check=True)
                            if dh == 0:
                                yield
                        nc.vector.tensor_scalar(
                            qk[f][:, 512 * c:512 * (c + 1)], pq[:],
                            bqk[:, f:f + 1], None, ALU.add)
                        yield

            return units()

        # ---------------- phase B: xT + v + q0/k0 ----------------
        with (
            tc.tile_pool(name="t_ps", bufs=2, space="PSUM") as t_ps,
            tc.tile_pool(name="v_ps", bufs=2, space="PSUM") as v_ps,
        ):
            f0 = qk_filler(0)

            def v_proj(t):
                pv = v_ps.tile([128, DG], F32, tag="pv")
                for d in range(ND):
                    nc.tensor.matmul(pv[:], xt[:, d, 128 * t:128 * (t + 1)],
                                     wv_sb[d][:], start=(d == 0),
                                     stop=(d == ND - 1))
                nc.vector.tensor_tensor(
                    vp[t][:].rearrange("p (h c) -> p h c", h=HPG)[:, :, 0:HD],
                    pv[:].rearrange("p (h c) -> p h c", h=HPG),
                    bv_bc[:].rearrange("p (h c) -> p h c", h=HPG),
                    ALU.add)

            for t in range(NT):
                if t < 4:
                    xtmp = xtmps[t]
                else:
                    xtmp = xtmp_p.tile([128, D], F32R, tag="xtmp", name="xtmp")
                    for hh in range(2):
                        nc.sync.dma_start(
                            xtmp[:, 512 * hh:512 * (hh + 1)],
                            x_d[128 * t:128 * (t + 1), 512 * hh:512 * (hh + 1)])
                # v of older tiles first: always-ready PE work that covers
                # the wait for x tile t's DMA; v of tiles 12-15 is deferred
                # into pair 0's attention window
                if 2 <= t and t - 2 < NT - 4:
                    v_proj(t - 2)
                for half in range(2):
                    tp = t_ps.tile([128, 512], F32R, tag="tp")
                    for dd in range(4):
                        d = 4 * half + dd
                        nc.tensor.transpose(
                            tp[:, 128 * dd:128 * (dd + 1)],
                            xtmp[:, 128 * d:128 * (d + 1)], identr[:])
                    nc.vector.tensor_copy(
                        xt[:, 4 * half:4 * half + 4, 128 * t:128 * (t + 1)],
                        tp[:].rearrange("p (a b) -> p a b", a=4))
                if t % 4 == 3:
                    # q0/k0 chunk (t-3)//4 just became computable: 4 units
                    for _ in range(4):
                        next(f0, None)
            for _ in f0:
                pass
        xtmp_es.close()

        # deferred v projection for tiles 12-15, emitted as pair-0 filler
        def v_filler():
            for t in range(NT - 4, NT):
                pv = qkproj_ps.tile([128, DG], F32, tag="pq", name="pv")
                for d in range(ND):
                    nc.tensor.matmul(pv[:], xt[:, d, 128 * t:128 * (t + 1)],
                                     wv_sb[d][:], start=(d == 0),
                                     stop=(d == ND - 1),
                                     skip_group_check=True)
                    if d == 3:
                        yield
                nc.vector.tensor_tensor(
                    vp[t][:].rearrange("p (h c) -> p h c", h=HPG)[:, :, 0:HD],
                    pv[:].rearrange("p (h c) -> p h c", h=HPG),
                    bv_bc[:].rearrange("p (h c) -> p h c", h=HPG),
                    ALU.add)
                yield

        # ---------------- attention ----------------
        def attention_pair(g, filler, rate=2, budget=None):
            qt = qk[g]
            kt = qk[NK + g]
            consumed = [0]

            def filler_step(n=1):
                for _ in range(n):
                    if next(filler, None) is not None or True:
                        consumed[0] += 1

            slot = 0
            for c in range(NC):
                JJ = 2 * c + 2
                pys = [py_ps.tile([VW, 512], F32, tag="py", name=f"py{g}{c}{x}")
                       for x in range(2)]

                def emit_avs(ent, last):
                    xi, jj, pt = ent
                    hl = 2 * g + xi
                    for half in range(2):
                        j = 2 * jj + half
                        # diagonal tiles: columns < 128m are fully masked and
                        # were never computed; skip them (psum cols keep their
                        # start-reset value from the first full-width matmul)
                        m = j - 4 * c
                        lo = 128 * m if m > 0 else 0
                        nc.tensor.matmul(
                            pys[xi][:, lo:512], vp[j][:, VW * hl:VW * (hl + 1)],
                            pt[:, 512 * half + lo:512 * (half + 1)],
                            start=(jj == 0 and half == 0),
                            stop=(last and half == 1),
                            skip_group_check=True)
                    if last:
                        qrow = xi * 64
                        ysl = yt[g][qrow:qrow + HD, 512 * c:512 * (c + 1)]
                        zr = zbc_p.tile([1, 512], F32, tag="zr", name="zr")
                        if g == NP - 1 and c == NC - 1:
                            # last chunk: ACT is done with exps; keep the
                            # critical normalize chain off the busy DVE queue
                            nc.scalar.activation(ysl, pys[xi][0:64, :], AF.Copy)
                            nc.scalar.activation(zr[:], pys[xi][64:65, :],
                                                 AF.Copy)
                        else:
                            nc.vector.tensor_copy(ysl, pys[xi][0:64, :])
                            nc.vector.tensor_copy(zr[:], pys[xi][64:65, :])
                        zrr = zbc_p.tile([1, 512], F32, tag="zrr", name="zrr")
                        nc.vector.reciprocal_approx_fast(zrr[:], zr[:])
                        zbc = zbc_p.tile([128, 512], F32, tag="zbc")
                        nc.gpsimd.partition_broadcast(
                            zbc[:, :], zrr[:], channels=128)
                        nc.vector.tensor_tensor(
                            ysl, ysl, zbc[qrow:qrow + HD, :], ALU.mult)

                prev = None
                for jj in range(JJ):
                    for xi, qrow in ((0, 0), (1, 64)):
                        st = s_ps.tile([128, 1024], F32, tag="s")
                        for half in range(2):
                            j = 2 * jj + half
                            m = j - 4 * c
                            diag = m >= 0
                            # columns < 128m are fully masked: don't compute
                            # them (the trimmed AV never reads them)
                            lo = 128 * m if diag else 0
                            nc.tensor.matmul(
                                st[:, 512 * half + lo:512 * (half + 1)],
                                kt[qrow:qrow + HD, 128 * j:128 * (j + 1)],
                                qt[qrow:qrow + HD,
                                   512 * c + lo:512 * (c + 1)],
                                start=True, stop=True,
                                skip_group_check=True)
                        pt = pt_p.tile([128, 1024], BF16, tag="pt")
                        if 2 * jj >= 4 * c:
                            # diagonal pair: exp only the computed column
                            # ranges of each half (the gap was never written)
                            lo0 = 128 * (2 * jj - 4 * c)
                            nc.scalar.activation(
                                pt[:, lo0:512], st[:, lo0:512],
                                AF.Exp, scale=0.125)
                            nc.scalar.activation(
                                pt[:, 512 + lo0 + 128:1024],
                                st[:, 512 + lo0 + 128:1024],
                                AF.Exp, scale=0.125)
                            # zero the above-diagonal triangle of the leading
                            # 128-col block of each half: keep where
                            # col_local - p >= 0 (tq >= tk)
                            for half in range(2):
                                lo = lo0 + 128 * half
                                nc.gpsimd.affine_select(
                                    out=pt[:, 512 * half + lo:
                                           512 * half + lo + 128],
                                    in_=pt[:, 512 * half + lo:
                                           512 * half + lo + 128],
                                    compare_op=ALU.is_ge, fill=0.0,
                                    base=0, channel_multiplier=-1,
                                    pattern=[[1, 128]])
                        else:
                            nc.scalar.activation(pt[:], st[:], AF.Exp,
                                                 scale=0.125)
                        if prev is not None:
                            emit_avs(prev, prev[1] == JJ - 1)
                            if budget is not None:
                                if consumed[0] < budget(c):
                                    filler_step()
                            elif slot % rate == 0:
                                filler_step()
                            slot += 1
                        prev = (xi, jj, pt)
                emit_avs(prev, True)
                filler_step()

        # full out projection emitted as PE filler inside pair 3's window,
        # chunk-gated: tile t needs yt[3] chunk t//4 normalized first.
        # Final rows land in ohalf and DMA out immediately.
        def out_filler():
            # last-4 tiles first: yt[0..2] partials (no chunk gating needed)
            for t in range(NT - 4, NT):
                for oc in range(2):
                    po = qkproj_ps.tile([128, 512], F32, tag="pq", name="pof")
                    for k in range(NK - 1):
                        nc.tensor.matmul(
                            po[:], yt[k][:, 128 * t:128 * (t + 1)],
                            wo_sb[k][:, 512 * oc:512 * (oc + 1)],
                            start=(k == 0), stop=(k == NK - 2),
                            skip_group_check=True)
                    nc.vector.tensor_tensor(
                        ohalf[:, t, 512 * oc:512 * (oc + 1)], po[:],
                        bo_bc[:, 512 * oc:512 * (oc + 1)], ALU.add)
                    yield
            # the rest: complete rows, gated on pair-3 chunk completion, DMA
            # out immediately
            for t in range(NT - 4):
                for oc in range(2):
                    po = qkproj_ps.tile([128, 512], F32, tag="pq", name="pof")
                    for k in range(NK):
                        nc.tensor.matmul(
                            po[:], yt[k][:, 128 * t:128 * (t + 1)],
                            wo_sb[k][:, 512 * oc:512 * (oc + 1)],
                            start=(k == 0), stop=(k == NK - 1),
                            skip_group_check=True)
                    osl = ohalf[:, t, 512 * oc:512 * (oc + 1)]
                    nc.vector.tensor_tensor(
                        osl, po[:], bo_bc[:, 512 * oc:512 * (oc + 1)],
                        ALU.add)
                    nc.sync.dma_start(
                        out_d[128 * t:128 * (t + 1),
                              512 * oc:512 * (oc + 1)], osl.bitcast(F32))
                    yield

        def out_flush():
            # add the yt[3] term to the last-4 tiles: re-add the partial row
            # via an identity matmul so the drain is a plain ACT copy (DVE
            # stays off the critical tail)
            for t in range(NT - 4, NT):
                for oc in range(2):
                    po = qkproj_ps.tile([128, 512], F32, tag="pq", name="pofl")
                    nc.tensor.matmul(
                        po[:], yt[NK - 1][:, 128 * t:128 * (t + 1)],
                        wo_sb[NK - 1][:, 512 * oc:512 * (oc + 1)],
                        start=True, stop=False, skip_group_check=True)
                    osl = ohalf[:, t, 512 * oc:512 * (oc + 1)]
                    nc.tensor.matmul(
                        po[:], identr[:], osl,
                        start=False, stop=True, skip_group_check=True)
                    ofl = ofl_p.tile([128, 512], F32, tag="ofl", name="ofl")
                    nc.scalar.activation(ofl[:], po[:], AF.Copy)
                    nc.sync.dma_start(
                        out_d[128 * t:128 * (t + 1),
                              512 * oc:512 * (oc + 1)], ofl[:])

        with (
            tc.tile_pool(name="s_ps", bufs=2, space="PSUM") as s_ps,
            tc.tile_pool(name="y_ps", bufs=2, space="PSUM") as py_ps,
        ):
            for g in range(NP):
                if g == 0:
                    from itertools import chain
                    filler = chain(v_filler(), qk_filler(1))
                    attention_pair(g, filler, rate=2)
                elif g < NP - 1:
                    filler = qk_filler(g + 1)
                    attention_pair(g, filler, rate=2)
                else:
                    filler = out_filler()
                    attention_pair(g, filler, budget=lambda c: 8 + 8 * c)
                for _ in filler:
                    pass
                if g == NP - 1:
                    out_flush()
                if g == 0:
                    wv_es.close()
                if g == NP - 2:
                    # all qk projections emitted; xt no longer needed
                    xt_es.close()
                    # half-done out rows live where xt was
                    oh_p = ctx.enter_context(tc.tile_pool(name="ohalf", bufs=1))
                    # f32r so the flush identity-matmul may re-add it
                    ohalf = oh_p.tile([128, NT, D], F32R, tag="ohalf",
                                      name="ohalf")
                    ofl_p = ctx.enter_context(
                        tc.tile_pool(name="oflsh", bufs=2))
                    # out-proj weights: DMA f32 staging, cast to bf16
                    wo_p = ctx.enter_context(tc.tile_pool(name="wo", bufs=1))
                    wost_p = ctx.enter_context(
                        tc.tile_pool(name="wost", bufs=2))
                    wo_sb = []
                    for k in range(NK):
                        wst = wost_p.tile([128, D], F32R, tag="wost",
                                          name="wost")
                        nc.sync.dma_start(wst[:], wo_d[128 * k:128 * (k + 1), :])
                        wt = wo_p.tile([128, D], BF16, tag=f"wo{k}",
                                       name=f"wot{k}")
                        nc.vector.tensor_copy(wt[:], wst[:].bitcast(F32))
                        wo_sb.append(wt)



def kernel(x, attn_mask, Wqkv, bqkv, Wout, bout):
    if "nc" not in _cached:
        _cached["nc"] = _build()
    nc = _cached["nc"]

    x = np.ascontiguousarray(x, dtype=np.float32)
    Wqkv = np.ascontiguousarray(Wqkv, dtype=np.float32)
    bqkv = np.ascontiguousarray(bqkv, dtype=np.float32)
    Wout = np.ascontiguousarray(Wout, dtype=np.float32)
    bout = np.ascontiguousarray(bout, dtype=np.float32)
    zeros_bo = np.zeros_like(bout)
    in_maps = []
    for b in range(B):
        for g in range(G):
            s = slice(g * DG, (g + 1) * DG)
            in_maps.append({
                "x": np.ascontiguousarray(x[b]),
                "wq": np.ascontiguousarray(Wqkv[:, g * DG:(g + 1) * DG]),
                "wk": np.ascontiguousarray(Wqkv[:, D + g * DG:D + (g + 1) * DG]),
                "wv": np.ascontiguousarray(Wqkv[:, 2 * D + g * DG:2 * D + (g + 1) * DG]),
                "wo": np.ascontiguousarray(Wout[s, :]),
                "bq": np.ascontiguousarray(bqkv[g * DG:(g + 1) * DG]),
                "bk": np.ascontiguousarray(bqkv[D + g * DG:D + (g + 1) * DG]),
                "bv": np.ascontiguousarray(bqkv[2 * D + g * DG:2 * D + (g + 1) * DG]),
                "bo": bout if g == 0 else zeros_bo,
            })

    trace = bool(int(os.environ.get("BASS_ATTN_TRACE", "0")))
    res = bass_utils.run_bass_kernel_spmd(
        nc, in_maps, core_ids=list(range(B * G)), trace=trace)
    _cached["last_result"] = res

    out = np.empty((B, T, D), dtype=np.float32)
    for b in range(B):
        out[b] = res.results[2 * b]["out"] + res.results[2 * b + 1]["out"]
    return out
